# revision 3
# baseline (speedup 1.0000x reference)
"""Causal self-attention (B=2, T=2048, C=1024, H=16) on 8 TRN2 NeuronCores.

Sharding: data parallel over batch (2) x tensor parallel over heads (4 groups
of 4 heads). Each core computes qkv + attention for its 4 heads of one batch,
normalized attention outputs are AllGathered within each batch group of 4
cores, and each core then computes a 256-column slice of the output
projection. The host concatenates the column slices (pure gather, no
reduction).
"""
import numpy as np
import ml_dtypes

import concourse.bass as bass
import concourse.tile as tile
from concourse import bacc, mybir
from concourse.bass_utils import run_bass_kernel_spmd

BF16 = ml_dtypes.bfloat16

B, T, C, H, D = 2, 2048, 1024, 16, 64
NCORES = 8
HPC = 4              # heads per core
FQK = 2 * HPC * D    # 512 rows of q+k per core
FV = HPC * D         # 256 rows of v per core
CT = C // 128        # 8 contraction tiles
TC5 = T // 512       # 4 t-chunks of 512
SB = T // 128        # 16 s-blocks of 128
SCALE = 1.0 / 8.0    # 1/sqrt(D)

_CACHE = {}


def _build_kernel():
    nc = bacc.Bacc("TRN2", target_bir_lowering=False, debug=False,
                   num_devices=NCORES)
    dt = mybir.dt
    f32, bf16, f32r = dt.float32, dt.bfloat16, dt.float32r

    xT = nc.dram_tensor("xT", [C, T], bf16, kind="ExternalInput").ap()
    wqkT = nc.dram_tensor("wqkT", [C, FQK], bf16, kind="ExternalInput").ap()
    wvT = nc.dram_tensor("wvT", [C, FV], bf16, kind="ExternalInput").ap()
    wpT = nc.dram_tensor("wpT", [C, FV], bf16, kind="ExternalInput").ap()
    bqk = nc.dram_tensor("bqk", [FQK, 1], f32, kind="ExternalInput").ap()
    bv = nc.dram_tensor("bv", [1, FV], bf16, kind="ExternalInput").ap()
    bp = nc.dram_tensor("bp", [1, FV], bf16, kind="ExternalInput").ap()
    maskb = nc.dram_tensor("maskb", [128, 896], bf16, kind="ExternalInput").ap()
    onesb = nc.dram_tensor("onesb", [1, 128], bf16, kind="ExternalInput").ap()
    onesr = nc.dram_tensor("onesr", [1, 64], f32r, kind="ExternalInput").ap()
    out = nc.dram_tensor("out", [T, FV], f32, kind="ExternalOutput").ap()

    with tile.TileContext(nc) as tc:
        with (
            tc.tile_pool(name="persist", bufs=1) as pp,
            tc.tile_pool(name="work", bufs=4) as wp,
            tc.tile_pool(name="attT", bufs=40) as ap_pool,
            tc.tile_pool(name="outsb", bufs=3) as op,
            tc.tile_pool(name="ps_qk", bufs=3, space="PSUM") as ps_qk,
            tc.tile_pool(name="ps_y", bufs=2, space="PSUM") as ps_y,
            tc.tile_pool(name="ps_bc", bufs=1, space="PSUM") as ps_bc,
            tc.tile_pool(name="ps_mm", bufs=2, space="PSUM") as ps_mm,
            tc.tile_pool(name="dram", bufs=1, space="DRAM") as dram,
        ):
            # ---- load constants / inputs to SBUF ----
            xT_s = pp.tile([128, CT, T], bf16, tag="xT")
            nc.sync.dma_start(xT_s[:], xT.rearrange("(n p) t -> p n t", p=128))
            wqk_s = pp.tile([128, CT, FQK], bf16, tag="wqk")
            nc.sync.dma_start(wqk_s[:], wqkT.rearrange("(n p) f -> p n f", p=128))
            wv_s = pp.tile([128, CT, FV], bf16, tag="wv")
            nc.sync.dma_start(wv_s[:], wvT.rearrange("(n p) f -> p n f", p=128))
            wp_s = pp.tile([128, CT, FV], bf16, tag="wp")
            nc.sync.dma_start(wp_s[:], wpT.rearrange("(n p) f -> p n f", p=128))
            bqk_s = pp.tile([128, 4], f32, tag="bqk")
            nc.sync.dma_start(bqk_s[:], bqk.rearrange("(n p) o -> p (n o)", p=128))
            bv_s = pp.tile([1, FV], bf16, tag="bv")
            nc.sync.dma_start(bv_s[:], bv[:])
            bp_s = pp.tile([1, FV], bf16, tag="bp")
            nc.sync.dma_start(bp_s[:], bp[:])
            mask_s = pp.tile([128, 896], bf16, tag="mask")
            nc.sync.dma_start(mask_s[:], maskb[:])
            ones_s = pp.tile([1, 128], bf16, tag="ones")
            nc.sync.dma_start(ones_s[:], onesb[:])
            onesr_s = pp.tile([1, 64], f32r, tag="onesr")
            nc.sync.dma_start(onesr_s[:], onesr[:])

            # ---- qkT = wqk^T.T @ x^T + b (transposed layout [f, t]) ----
            qkT_s = [pp.tile([128, T], bf16, tag=f"qkT{fc}", name=f"qkT{fc}")
                     for fc in range(4)]
            for fc in range(4):
                for t5 in range(TC5):
                    ps = ps_qk.tile([128, 512], f32, tag="qk")
                    for ci in range(CT):
                        nc.tensor.matmul(
                            ps[:],
                            wqk_s[:, ci, fc * 128:(fc + 1) * 128],
                            xT_s[:, ci, t5 * 512:(t5 + 1) * 512],
                            start=(ci == 0), stop=(ci == CT - 1),
                        )
                    nc.vector.tensor_scalar_add(
                        qkT_s[fc][:, t5 * 512:(t5 + 1) * 512], ps[:],
                        bqk_s[:, fc:fc + 1],
                    )

            # ---- v natural [t, 4*65] with ones column per head ----
            vaug = pp.tile([128, SB, HPC * 65], bf16, tag="vaug")
            nc.vector.memset(vaug[:], 1.0)
            for tb in range(SB):
                ps = ps_mm.tile([128, FV], f32, tag="mm")
                nc.tensor.matmul(ps[:], ones_s[0:1, 0:128], bv_s[0:1, :],
                                 start=True, stop=False)
                for ci in range(CT):
                    nc.tensor.matmul(
                        ps[:],
                        xT_s[:, ci, tb * 128:(tb + 1) * 128],
                        wv_s[:, ci, :],
                        start=False, stop=(ci == CT - 1),
                    )
                dst = vaug[:, tb, :].rearrange("p (h x) -> p h x", h=HPC)[:, :, 0:64]
                src = ps[:].rearrange("p (h x) -> p h x", h=HPC)
                nc.vector.tensor_copy(dst, src)

            # ---- attention per head pair ----
            ynorm = [pp.tile([64, T], bf16, tag=f"ynorm{h}", name=f"ynorm{h}")
                     for h in range(HPC)]
            for pair in range(2):
                q_fc, k_fc = pair, 2 + pair
                for t5 in range(TC5):
                    live = 4 * (t5 + 1)
                    att = {}
                    for sb in range(live):
                        for hh in range(2):
                            lo, hi = 64 * hh, 64 * (hh + 1)
                            ps = ps_qk.tile([128, 512], f32, tag="qk")
                            nc.tensor.matmul(
                                ps[:],
                                qkT_s[k_fc][lo:hi, sb * 128:(sb + 1) * 128],
                                qkT_s[q_fc][lo:hi, t5 * 512:(t5 + 1) * 512],
                                start=True, stop=True,
                            )
                            a = ap_pool.tile([128, 512], bf16, tag="attT")
                            nc.scalar.activation(
                                a[:], ps[:],
                                mybir.ActivationFunctionType.Exp, scale=SCALE,
                            )
                            off = sb * 128 - t5 * 512
                            if off >= 0:  # diagonal block: apply causal mask
                                mslice = mask_s[:, 384 - off:896 - off]
                                nc.vector.tensor_mul(a[:], a[:], mslice)
                            att[(sb, hh)] = a
                    for hh in range(2):
                        h = pair * 2 + hh
                        yps = ps_y.tile([65, 512], f32, tag="y")
                        for sb in range(live):
                            nc.tensor.matmul(
                                yps[:],
                                vaug[:, sb, h * 65:(h + 1) * 65],
                                att[(sb, hh)][:],
                                start=(sb == 0), stop=(sb == live - 1),
                            )
                        # denominator (row 64) -> broadcast -> reciprocal -> mul
                        den = wp.tile([1, 512], f32r, tag="den")
                        nc.scalar.activation(
                            den[:], yps[64:65, :],
                            mybir.ActivationFunctionType.Identity,
                        )
                        bc = ps_bc.tile([64, 512], f32, tag="bc")
                        nc.tensor.matmul(bc[:], onesr_s[0:1, :], den[:],
                                         start=True, stop=True)
                        r = wp.tile([64, 512], f32, tag="recip")
                        nc.vector.reciprocal(r[:], bc[:])
                        nc.vector.tensor_mul(
                            ynorm[h][:, t5 * 512:(t5 + 1) * 512],
                            yps[0:64, :], r[:],
                        )

            # ---- AllGather normalized y^T across the 4 cores of the batch ----
            ag_in = dram.tile([FV, T], bf16)
            ag_out = dram.tile([4 * FV, T], bf16)
            for h in range(HPC):
                nc.sync.dma_start(ag_in[h * 64:(h + 1) * 64, :], ynorm[h][:])
            nc.gpsimd.collective_compute(
                "AllGather", mybir.AluOpType.bypass,
                replica_groups=[[0, 1, 2, 3], [4, 5, 6, 7]],
                ins=[ag_in[:].opt()], outs=[ag_out[:].opt()],
            )
            yfT = pp.tile([128, CT, T], bf16, tag="yfT")
            nc.sync.dma_start(yfT[:], ag_out[:].rearrange("(n p) t -> p n t", p=128))

            # ---- out column slice: out[t, 256] = y @ wp^T slice + bias ----
            for tb in range(SB):
                ps = ps_mm.tile([128, FV], f32, tag="mm")
                nc.tensor.matmul(ps[:], ones_s[0:1, 0:128], bp_s[0:1, :],
                                 start=True, stop=False)
                for ci in range(CT):
                    nc.tensor.matmul(
                        ps[:],
                        yfT[:, ci, tb * 128:(tb + 1) * 128],
                        wp_s[:, ci, :],
                        start=False, stop=(ci == CT - 1),
                    )
                osb = op.tile([128, FV], f32, tag="osb")
                nc.vector.tensor_copy(osb[:], ps[:])
                nc.sync.dma_start(out[tb * 128:(tb + 1) * 128, :], osb[:])

    nc.compile()
    return nc


def _shard_inputs(x, w_attn, b_attn, w_proj, b_proj):
    mask = np.zeros((128, 896), dtype=BF16)
    for p in range(128):
        mask[p, 384 + p:] = 1.0
    ones_b = np.ones((1, 128), dtype=BF16)
    ones_r = np.ones((1, 64), dtype=np.float32)

    in_maps = []
    for core in range(NCORES):
        b, hg = core // 4, core % 4
        r0 = hg * HPC * D          # first q/k/v row offset within each 1024
        r1 = r0 + HPC * D
        wqk = np.concatenate([w_attn[r0:r1, :], w_attn[C + r0:C + r1, :]], 0)
        in_maps.append({
            "xT": np.ascontiguousarray(x[b].T).astype(BF16),
            "wqkT": np.ascontiguousarray(wqk.T).astype(BF16),
            "wvT": np.ascontiguousarray(w_attn[2 * C + r0:2 * C + r1, :].T).astype(BF16),
            "wpT": np.ascontiguousarray(w_proj[r0:r1, :].T).astype(BF16),
            "bqk": np.concatenate([b_attn[r0:r1], b_attn[C + r0:C + r1]])
                     .reshape(FQK, 1).astype(np.float32),
            "bv": b_attn[2 * C + r0:2 * C + r1].reshape(1, FV).astype(BF16),
            "bp": b_proj[r0:r1].reshape(1, FV).astype(BF16),
            "maskb": mask,
            "onesb": ones_b,
            "onesr": ones_r,
        })
    return in_maps


def kernel(x, w_attn, b_attn, w_proj, b_proj, _trace=False, _trace_kwargs=None):
    x = np.asarray(x, dtype=np.float32)
    w_attn = np.asarray(w_attn, dtype=np.float32)
    b_attn = np.asarray(b_attn, dtype=np.float32)
    w_proj = np.asarray(w_proj, dtype=np.float32)
    b_proj = np.asarray(b_proj, dtype=np.float32)

    if "nc" not in _CACHE:
        _CACHE["nc"] = _build_kernel()
    nc = _CACHE["nc"]

    in_maps = _shard_inputs(x, w_attn, b_attn, w_proj, b_proj)
    res = run_bass_kernel_spmd(nc, in_maps, core_ids=list(range(NCORES)),
                               trace=_trace, **(_trace_kwargs or {}))
    _CACHE["last_result"] = res

    out = np.empty((B, T, C), dtype=np.float32)
    for core in range(NCORES):
        b, hg = core // 4, core % 4
        out[b, :, hg * FV:(hg + 1) * FV] = res.results[core]["out"]
    return out


# revision 7
# speedup vs baseline: 1.6781x; 1.6781x over previous
"""Causal self-attention (B=2, T=2048, C=1024, H=16) on 8 TRN2 NeuronCores.

Sharding: data parallel over batch (2) x tensor parallel over heads (4 groups
of 4 heads). Each core computes qkv + attention for its 4 heads of one batch,
normalized attention outputs are AllGathered (chunked over t, overlapped with
attention) within each batch group of 4 cores, and each core then computes a
256-column slice of the output projection. The host concatenates the column
slices (pure gather, no reduction).
"""
import numpy as np
import ml_dtypes

import concourse.bass as bass
import concourse.tile as tile
from concourse import bacc, mybir
from concourse.bass_utils import run_bass_kernel_spmd

BF16 = ml_dtypes.bfloat16

B, T, C, H, D = 2, 2048, 1024, 16, 64
NCORES = 8
HPC = 4              # heads per core
FQK = 2 * HPC * D    # 512 rows of q+k per core
FV = HPC * D         # 256 rows of v per core
CT = C // 128        # 8 contraction tiles
TC5 = T // 512       # 4 t-chunks of 512
SB = T // 128        # 16 s-blocks of 128
SCALE = 1.0 / 8.0    # 1/sqrt(D)

_CACHE = {}


def _build_kernel():
    nc = bacc.Bacc("TRN2", target_bir_lowering=False, debug=False,
                   num_devices=NCORES)
    dt = mybir.dt
    f32, bf16 = dt.float32, dt.bfloat16

    xT = nc.dram_tensor("xT", [C, T], bf16, kind="ExternalInput").ap()
    wqkT = nc.dram_tensor("wqkT", [C, FQK], bf16, kind="ExternalInput").ap()
    wvT = nc.dram_tensor("wvT", [C, FV], bf16, kind="ExternalInput").ap()
    wpT = nc.dram_tensor("wpT", [C, FV], bf16, kind="ExternalInput").ap()
    bqk = nc.dram_tensor("bqk", [FQK, 1], f32, kind="ExternalInput").ap()
    bv = nc.dram_tensor("bv", [1, FV], bf16, kind="ExternalInput").ap()
    bp = nc.dram_tensor("bp", [1, FV], bf16, kind="ExternalInput").ap()
    maskb = nc.dram_tensor("maskb", [128, 896], bf16, kind="ExternalInput").ap()
    onesr = nc.dram_tensor("onesr", [1, 64], mybir.dt.float32r,
                           kind="ExternalInput").ap()
    out = nc.dram_tensor("out", [T, FV], f32, kind="ExternalOutput").ap()

    with tile.TileContext(nc) as tc:
        with (
            tc.tile_pool(name="persist", bufs=1) as pp,
            tc.tile_pool(name="work", bufs=4) as wp,
            tc.tile_pool(name="attT", bufs=6) as ap_pool,
            tc.tile_pool(name="outsb", bufs=3) as op,
            tc.tile_pool(name="ps_qk", bufs=2, space="PSUM") as ps_qk,
            tc.tile_pool(name="ps_y", bufs=2, space="PSUM") as ps_y,
            tc.tile_pool(name="ps_bc", bufs=1, space="PSUM") as ps_bc,
            tc.tile_pool(name="ps_mm", bufs=1, space="PSUM") as ps_mm,
            tc.tile_pool(name="dram", bufs=1, space="DRAM") as dram,
        ):
            # ---- load constants / inputs to SBUF (split for pipelining) ----
            wqk_s = pp.tile([128, CT, FQK], bf16, tag="wqk")
            xT_s = pp.tile([128, CT, T], bf16, tag="xT")
            wv_s = pp.tile([128, CT, FV], bf16, tag="wv")
            xTr = xT.rearrange("(n p) t -> p n t", p=128)
            wqkr = wqkT.rearrange("(n p) f -> p n f", p=128)
            wvr = wvT.rearrange("(n p) f -> p n f", p=128)
            for ci in range(CT):
                nc.sync.dma_start(wqk_s[:, ci, :], wqkr[:, ci, :])
                nc.sync.dma_start(xT_s[:, ci, :], xTr[:, ci, :])
                nc.sync.dma_start(wv_s[:, ci, :], wvr[:, ci, :])
            wp_s = pp.tile([128, CT, FV], bf16, tag="wp")
            nc.sync.dma_start(wp_s[:], wpT.rearrange("(n p) f -> p n f", p=128))
            bqk_s = pp.tile([128, 4], f32, tag="bqk")
            nc.sync.dma_start(bqk_s[:], bqk.rearrange("(n p) o -> p (n o)", p=128))
            bv_s = pp.tile([1, FV], bf16, tag="bv")
            nc.sync.dma_start(bv_s[:], bv[:])
            bp_s = pp.tile([1, FV], bf16, tag="bp")
            nc.sync.dma_start(bp_s[:], bp[:])
            mask_s = pp.tile([128, 896], bf16, tag="mask")
            nc.sync.dma_start(mask_s[:], maskb[:])
            ones16 = pp.tile([1, 128], bf16, tag="ones16")
            nc.vector.memset(ones16[:], 1.0)
            ones32 = pp.tile([1, 64], mybir.dt.float32r, tag="ones32")
            nc.sync.dma_start(ones32[:], onesr[:])

            # ---- qkT = wqk^T.T @ x^T + b (transposed layout [f, t]) ----
            qkT_s = [pp.tile([128, T], bf16, tag=f"qkT{fc}", name=f"qkT{fc}")
                     for fc in range(4)]
            for t5 in range(TC5):
                for fc in range(4):
                    ps = ps_qk.tile([128, 512], f32, tag="qk", name="ps_qkv")
                    for ci in range(CT):
                        nc.tensor.matmul(
                            ps[:],
                            wqk_s[:, ci, fc * 128:(fc + 1) * 128],
                            xT_s[:, ci, t5 * 512:(t5 + 1) * 512],
                            start=(ci == 0), stop=(ci == CT - 1),
                        )
                    nc.vector.tensor_scalar_add(
                        qkT_s[fc][:, t5 * 512:(t5 + 1) * 512], ps[:],
                        bqk_s[:, fc:fc + 1],
                    )

            # ---- v natural [t, 4*65] with ones column per head ----
            vaug = pp.tile([128, SB, HPC * 65], bf16, tag="vaug")
            nc.vector.memset(vaug[:], 1.0)
            for tb in range(SB):
                ps = ps_mm.tile([128, FV], f32, tag="mm", name="ps_v")
                nc.tensor.matmul(ps[:], ones16[0:1, :], bv_s[0:1, :],
                                 start=True, stop=False)
                for ci in range(CT):
                    nc.tensor.matmul(
                        ps[:],
                        xT_s[:, ci, tb * 128:(tb + 1) * 128],
                        wv_s[:, ci, :],
                        start=False, stop=(ci == CT - 1),
                    )
                dst = vaug[:, tb, :].rearrange("p (h x) -> p h x", h=HPC)[:, :, 0:64]
                src = ps[:].rearrange("p (h x) -> p h x", h=HPC)
                nc.vector.tensor_copy(dst, src)

            # ---- attention, t-chunk major; AG + proj pipelined behind it ----
            ag_in, ag_out, yf = {}, {}, {}

            def proj_chunk(t5):
                yft = yf[t5]
                for tq in range(4):
                    tb = t5 * 4 + tq
                    pso = ps_mm.tile([128, FV], f32, tag="mm", name="ps_o")
                    nc.tensor.matmul(pso[:], ones16[0:1, :], bp_s[0:1, :],
                                     start=True, stop=False)
                    for ci in range(CT):
                        nc.tensor.matmul(
                            pso[:],
                            yft[:, ci, tq * 128:(tq + 1) * 128],
                            wp_s[:, ci, :],
                            start=False, stop=(ci == CT - 1),
                        )
                    osb = op.tile([128, FV], f32, tag="osb", name="osb")
                    nc.vector.tensor_copy(osb[:], pso[:])
                    nc.sync.dma_start(out[tb * 128:(tb + 1) * 128, :], osb[:])

            for t5 in range(TC5):
                live = 4 * (t5 + 1)
                ag_in[t5] = dram.tile([FV, 512], bf16, tag=f"agin{t5}",
                                      name=f"agin{t5}")
                ag_out[t5] = dram.tile([4 * FV, 512], bf16, tag=f"agout{t5}",
                                       name=f"agout{t5}")
                for pair in range(2):
                    q_fc, k_fc = pair, 2 + pair
                    ypsA = ps_y.tile([65, 512], f32, tag="y", name="ypsA")
                    ypsB = ps_y.tile([65, 512], f32, tag="y", name="ypsB")
                    for sb in range(live):
                        ps = ps_qk.tile([128, 1024], f32, tag="qk", name="ps_s")
                        for hh in range(2):
                            lo, hi = 64 * hh, 64 * (hh + 1)
                            nc.tensor.matmul(
                                ps[:, hh * 512:(hh + 1) * 512],
                                qkT_s[k_fc][lo:hi, sb * 128:(sb + 1) * 128],
                                qkT_s[q_fc][lo:hi, t5 * 512:(t5 + 1) * 512],
                                start=True, stop=True,
                            )
                        a = ap_pool.tile([128, 1024], bf16, tag="attT",
                                         name="attT")
                        nc.scalar.activation(
                            a[:], ps[:],
                            mybir.ActivationFunctionType.Exp, scale=SCALE,
                        )
                        off = sb * 128 - t5 * 512
                        if off >= 0:  # diagonal block: apply causal mask
                            msl = mask_s[:, 384 - off:896 - off]
                            nc.vector.tensor_mul(a[:, 0:512], a[:, 0:512], msl)
                            nc.vector.tensor_mul(a[:, 512:1024], a[:, 512:1024], msl)
                        for hh, yps in ((0, ypsA), (1, ypsB)):
                            h = pair * 2 + hh
                            nc.tensor.matmul(
                                yps[:],
                                vaug[:, sb, h * 65:(h + 1) * 65],
                                a[:, hh * 512:(hh + 1) * 512],
                                start=(sb == 0), stop=(sb == live - 1),
                            )
                    # normalize: y / denom (denom = row 64 via ones column)
                    for hh, yps in ((0, ypsA), (1, ypsB)):
                        h = pair * 2 + hh
                        den = wp.tile([1, 512], mybir.dt.float32r, tag="den",
                                      name="den")
                        nc.vector.tensor_copy(den[:], yps[64:65, :])
                        bc = ps_bc.tile([64, 512], f32, tag="bc", name="bc")
                        nc.tensor.matmul(bc[:], ones32[0:1, :], den[:],
                                         start=True, stop=True)
                        r = wp.tile([64, 512], f32, tag="recip", name="recip")
                        nc.vector.reciprocal_approx_fast(r[:], bc[:])
                        yn = wp.tile([64, 512], bf16, tag="yn", name="yn")
                        nc.vector.tensor_mul(yn[:], yps[0:64, :], r[:])
                        nc.sync.dma_start(ag_in[t5][h * 64:(h + 1) * 64, :], yn[:])
                nc.gpsimd.collective_compute(
                    "AllGather", mybir.AluOpType.bypass,
                    replica_groups=[[0, 1, 2, 3], [4, 5, 6, 7]],
                    ins=[ag_in[t5][:].opt()], outs=[ag_out[t5][:].opt()],
                )
                yf[t5] = pp.tile([128, CT, 512], bf16, tag="yf", bufs=2,
                                 name=f"yf{t5}")
                nc.sync.dma_start(
                    yf[t5][:], ag_out[t5][:].rearrange("(n p) t -> p n t", p=128))
                if t5 >= 2:
                    proj_chunk(t5 - 2)
            proj_chunk(2)
            proj_chunk(3)

    nc.compile()
    return nc


def _shard_inputs(x, w_attn, b_attn, w_proj, b_proj):
    mask = np.zeros((128, 896), dtype=BF16)
    for p in range(128):
        mask[p, 384 + p:] = 1.0

    in_maps = []
    for core in range(NCORES):
        b, hg = core // 4, core % 4
        r0 = hg * HPC * D          # first q/k/v row offset within each 1024
        r1 = r0 + HPC * D
        wqk = np.concatenate([w_attn[r0:r1, :], w_attn[C + r0:C + r1, :]], 0)
        in_maps.append({
            "xT": np.ascontiguousarray(x[b].T).astype(BF16),
            "wqkT": np.ascontiguousarray(wqk.T).astype(BF16),
            "wvT": np.ascontiguousarray(w_attn[2 * C + r0:2 * C + r1, :].T).astype(BF16),
            "wpT": np.ascontiguousarray(w_proj[r0:r1, :].T).astype(BF16),
            "bqk": np.concatenate([b_attn[r0:r1], b_attn[C + r0:C + r1]])
                     .reshape(FQK, 1).astype(np.float32),
            "bv": b_attn[2 * C + r0:2 * C + r1].reshape(1, FV).astype(BF16),
            "bp": b_proj[r0:r1].reshape(1, FV).astype(BF16),
            "maskb": mask,
            "onesr": np.ones((1, 64), dtype=np.float32),
        })
    return in_maps


def kernel(x, w_attn, b_attn, w_proj, b_proj, _trace=False, _trace_kwargs=None):
    x = np.asarray(x, dtype=np.float32)
    w_attn = np.asarray(w_attn, dtype=np.float32)
    b_attn = np.asarray(b_attn, dtype=np.float32)
    w_proj = np.asarray(w_proj, dtype=np.float32)
    b_proj = np.asarray(b_proj, dtype=np.float32)

    if "nc" not in _CACHE:
        _CACHE["nc"] = _build_kernel()
    nc = _CACHE["nc"]

    in_maps = _shard_inputs(x, w_attn, b_attn, w_proj, b_proj)
    res = run_bass_kernel_spmd(nc, in_maps, core_ids=list(range(NCORES)),
                               trace=_trace, **(_trace_kwargs or {}))
    _CACHE["last_result"] = res

    out = np.empty((B, T, C), dtype=np.float32)
    for core in range(NCORES):
        b, hg = core // 4, core % 4
        out[b, :, hg * FV:(hg + 1) * FV] = res.results[core]["out"]
    return out


# revision 8
# speedup vs baseline: 1.6980x; 1.0119x over previous
"""Causal self-attention (B=2, T=2048, C=1024, H=16) on 8 TRN2 NeuronCores.

Sharding: data parallel over batch (2) x tensor parallel over heads (4 groups
of 4 heads). Each core computes qkv + attention for its 4 heads of one batch,
normalized attention outputs are AllGathered (chunked over t, overlapped with
attention) within each batch group of 4 cores, and each core then computes a
256-column slice of the output projection. The host concatenates the column
slices (pure gather, no reduction).
"""
import numpy as np
import ml_dtypes

import concourse.bass as bass
import concourse.tile as tile
from concourse import bacc, mybir
from concourse.bass_utils import run_bass_kernel_spmd

BF16 = ml_dtypes.bfloat16

B, T, C, H, D = 2, 2048, 1024, 16, 64
NCORES = 8
HPC = 4              # heads per core
FQK = 2 * HPC * D    # 512 rows of q+k per core
FV = HPC * D         # 256 rows of v per core
CT = C // 128        # 8 contraction tiles
TC5 = T // 512       # 4 t-chunks of 512
SB = T // 128        # 16 s-blocks of 128
SCALE = 1.0 / 8.0    # 1/sqrt(D)

_CACHE = {}


def _build_kernel():
    nc = bacc.Bacc("TRN2", target_bir_lowering=False, debug=False,
                   num_devices=NCORES)
    dt = mybir.dt
    f32, bf16 = dt.float32, dt.bfloat16

    xT = nc.dram_tensor("xT", [C, T], bf16, kind="ExternalInput").ap()
    wqkT = nc.dram_tensor("wqkT", [C, FQK], bf16, kind="ExternalInput").ap()
    wvT = nc.dram_tensor("wvT", [C, FV], bf16, kind="ExternalInput").ap()
    wpT = nc.dram_tensor("wpT", [C, FV], bf16, kind="ExternalInput").ap()
    bqk = nc.dram_tensor("bqk", [FQK, 1], f32, kind="ExternalInput").ap()
    bv = nc.dram_tensor("bv", [1, FV], bf16, kind="ExternalInput").ap()
    bp = nc.dram_tensor("bp", [1, FV], bf16, kind="ExternalInput").ap()
    maskb = nc.dram_tensor("maskb", [128, 896], bf16, kind="ExternalInput").ap()
    onesr = nc.dram_tensor("onesr", [1, 64], mybir.dt.float32r,
                           kind="ExternalInput").ap()
    out = nc.dram_tensor("out", [T, FV], f32, kind="ExternalOutput").ap()

    with tile.TileContext(nc) as tc:
        with (
            tc.tile_pool(name="persist", bufs=1) as pp,
            tc.tile_pool(name="work", bufs=4) as wp,
            tc.tile_pool(name="attT", bufs=6) as ap_pool,
            tc.tile_pool(name="outsb", bufs=3) as op,
            tc.tile_pool(name="ps_qk", bufs=2, space="PSUM") as ps_qk,
            tc.tile_pool(name="ps_y", bufs=2, space="PSUM") as ps_y,
            tc.tile_pool(name="ps_bc", bufs=1, space="PSUM") as ps_bc,
            tc.tile_pool(name="ps_mm", bufs=1, space="PSUM") as ps_mm,
            tc.tile_pool(name="dram", bufs=1, space="DRAM") as dram,
        ):
            # ---- load constants / inputs to SBUF (split for pipelining) ----
            wqk_s = pp.tile([128, CT, FQK], bf16, tag="wqk")
            xT_s = pp.tile([128, CT, T], bf16, tag="xT")
            wv_s = pp.tile([128, CT, FV], bf16, tag="wv")
            xTr = xT.rearrange("(n p) t -> p n t", p=128)
            wqkr = wqkT.rearrange("(n p) f -> p n f", p=128)
            wvr = wvT.rearrange("(n p) f -> p n f", p=128)
            for ci in range(CT):
                nc.sync.dma_start(wqk_s[:, ci, :], wqkr[:, ci, :])
                nc.sync.dma_start(xT_s[:, ci, :], xTr[:, ci, :])
                nc.sync.dma_start(wv_s[:, ci, :], wvr[:, ci, :])
            wp_s = pp.tile([128, CT, FV], bf16, tag="wp")
            nc.sync.dma_start(wp_s[:], wpT.rearrange("(n p) f -> p n f", p=128))
            bqk_s = pp.tile([128, 4], f32, tag="bqk")
            nc.sync.dma_start(bqk_s[:], bqk.rearrange("(n p) o -> p (n o)", p=128))
            bv_s = pp.tile([1, FV], bf16, tag="bv")
            nc.sync.dma_start(bv_s[:], bv[:])
            bp_s = pp.tile([1, FV], bf16, tag="bp")
            nc.sync.dma_start(bp_s[:], bp[:])
            mask_s = pp.tile([128, 896], bf16, tag="mask")
            nc.sync.dma_start(mask_s[:], maskb[:])
            ones16 = pp.tile([1, 128], bf16, tag="ones16")
            nc.vector.memset(ones16[:], 1.0)
            ones32 = pp.tile([1, 64], mybir.dt.float32r, tag="ones32")
            nc.sync.dma_start(ones32[:], onesr[:])
            # broadcast bias rows to all 128 partitions once (K=1 matmuls)
            bv_bc = pp.tile([128, FV], f32, tag="bv_bc")
            bp_bc = pp.tile([128, FV], f32, tag="bp_bc")
            for row, bc_t in ((bv_s, bv_bc), (bp_s, bp_bc)):
                psb = ps_mm.tile([128, FV], f32, tag="mm", name="ps_bias")
                nc.tensor.matmul(psb[:], ones16[0:1, :], row[0:1, :],
                                 start=True, stop=True)
                nc.vector.tensor_copy(bc_t[:], psb[:])

            # ---- qkT / v chunk emitters (interleaved into attention loop) ----
            qkT_s = [pp.tile([128, T], bf16, tag=f"qkT{fc}", name=f"qkT{fc}")
                     for fc in range(4)]
            vaug = pp.tile([128, SB, HPC * 65], bf16, tag="vaug")
            nc.vector.memset(vaug[:], 1.0)

            def qkT_chunk(t5):
                for fc in range(4):
                    ps = ps_qk.tile([128, 512], f32, tag="qk", name="ps_qkv")
                    for ci in range(CT):
                        nc.tensor.matmul(
                            ps[:],
                            wqk_s[:, ci, fc * 128:(fc + 1) * 128],
                            xT_s[:, ci, t5 * 512:(t5 + 1) * 512],
                            start=(ci == 0), stop=(ci == CT - 1),
                        )
                    nc.vector.tensor_scalar_add(
                        qkT_s[fc][:, t5 * 512:(t5 + 1) * 512], ps[:],
                        bqk_s[:, fc:fc + 1],
                    )

            def v_chunk(t5):
                for tb in range(4 * t5, 4 * t5 + 4):
                    ps = ps_mm.tile([128, FV], f32, tag="mm", name="ps_v")
                    for ci in range(CT):
                        nc.tensor.matmul(
                            ps[:],
                            xT_s[:, ci, tb * 128:(tb + 1) * 128],
                            wv_s[:, ci, :],
                            start=(ci == 0), stop=(ci == CT - 1),
                        )
                    dst = vaug[:, tb, :].rearrange("p (h x) -> p h x", h=HPC)[:, :, 0:64]
                    src = ps[:].rearrange("p (h x) -> p h x", h=HPC)
                    bias = bv_bc[:].rearrange("p (h x) -> p h x", h=HPC)
                    nc.vector.scalar_tensor_tensor(
                        dst, src, 1.0, bias,
                        op0=mybir.AluOpType.mult, op1=mybir.AluOpType.add,
                    )

            # ---- attention, t-chunk major; AG + proj pipelined behind it ----
            ag_in, ag_out, yf = {}, {}, {}

            def proj_chunk(t5):
                yft = yf[t5]
                for tq in range(4):
                    tb = t5 * 4 + tq
                    pso = ps_mm.tile([128, FV], f32, tag="mm", name="ps_o")
                    for ci in range(CT):
                        nc.tensor.matmul(
                            pso[:],
                            yft[:, ci, tq * 128:(tq + 1) * 128],
                            wp_s[:, ci, :],
                            start=(ci == 0), stop=(ci == CT - 1),
                        )
                    osb = op.tile([128, FV], f32, tag="osb", name="osb")
                    nc.vector.tensor_add(osb[:], pso[:], bp_bc[:])
                    nc.sync.dma_start(out[tb * 128:(tb + 1) * 128, :], osb[:])

            qkT_chunk(0)
            v_chunk(0)
            for t5 in range(TC5):
                if t5 + 1 < TC5:
                    qkT_chunk(t5 + 1)
                    v_chunk(t5 + 1)
                live = 4 * (t5 + 1)
                ag_in[t5] = dram.tile([FV, 512], bf16, tag=f"agin{t5}",
                                      name=f"agin{t5}")
                ag_out[t5] = dram.tile([4 * FV, 512], bf16, tag=f"agout{t5}",
                                       name=f"agout{t5}")
                for pair in range(2):
                    q_fc, k_fc = pair, 2 + pair
                    ypsA = ps_y.tile([65, 512], f32, tag="y", name="ypsA")
                    ypsB = ps_y.tile([65, 512], f32, tag="y", name="ypsB")
                    for sb in range(live):
                        ps = ps_qk.tile([128, 1024], f32, tag="qk", name="ps_s")
                        for hh in range(2):
                            lo, hi = 64 * hh, 64 * (hh + 1)
                            nc.tensor.matmul(
                                ps[:, hh * 512:(hh + 1) * 512],
                                qkT_s[k_fc][lo:hi, sb * 128:(sb + 1) * 128],
                                qkT_s[q_fc][lo:hi, t5 * 512:(t5 + 1) * 512],
                                start=True, stop=True,
                            )
                        a = ap_pool.tile([128, 1024], bf16, tag="attT",
                                         name="attT")
                        nc.scalar.activation(
                            a[:], ps[:],
                            mybir.ActivationFunctionType.Exp, scale=SCALE,
                        )
                        off = sb * 128 - t5 * 512
                        if off >= 0:  # diagonal block: mask cols [0, off+128)
                            w = off + 128
                            msl = mask_s[:, 384 - off:384 - off + w]
                            nc.vector.tensor_mul(a[:, 0:w], a[:, 0:w], msl)
                            nc.vector.tensor_mul(a[:, 512:512 + w],
                                                 a[:, 512:512 + w], msl)
                        for hh, yps in ((0, ypsA), (1, ypsB)):
                            h = pair * 2 + hh
                            nc.tensor.matmul(
                                yps[:],
                                vaug[:, sb, h * 65:(h + 1) * 65],
                                a[:, hh * 512:(hh + 1) * 512],
                                start=(sb == 0), stop=(sb == live - 1),
                            )
                    # normalize: y / denom (denom = row 64 via ones column)
                    for hh, yps in ((0, ypsA), (1, ypsB)):
                        h = pair * 2 + hh
                        den = wp.tile([1, 512], mybir.dt.float32r, tag="den",
                                      name="den")
                        nc.vector.tensor_copy(den[:], yps[64:65, :])
                        bc = ps_bc.tile([64, 512], f32, tag="bc", name="bc")
                        nc.tensor.matmul(bc[:], ones32[0:1, :], den[:],
                                         start=True, stop=True)
                        r = wp.tile([64, 512], f32, tag="recip", name="recip")
                        nc.vector.reciprocal_approx_fast(r[:], bc[:])
                        yn = wp.tile([64, 512], bf16, tag="yn", name="yn")
                        nc.vector.tensor_mul(yn[:], yps[0:64, :], r[:])
                        nc.sync.dma_start(ag_in[t5][h * 64:(h + 1) * 64, :], yn[:])
                nc.gpsimd.collective_compute(
                    "AllGather", mybir.AluOpType.bypass,
                    replica_groups=[[0, 1, 2, 3], [4, 5, 6, 7]],
                    ins=[ag_in[t5][:].opt()], outs=[ag_out[t5][:].opt()],
                )
                yf[t5] = pp.tile([128, CT, 512], bf16, tag="yf", bufs=2,
                                 name=f"yf{t5}")
                nc.sync.dma_start(
                    yf[t5][:], ag_out[t5][:].rearrange("(n p) t -> p n t", p=128))
                if t5 >= 2:
                    proj_chunk(t5 - 2)
            proj_chunk(2)
            proj_chunk(3)

    nc.compile()
    return nc


def _shard_inputs(x, w_attn, b_attn, w_proj, b_proj):
    mask = np.zeros((128, 896), dtype=BF16)
    for p in range(128):
        mask[p, 384 + p:] = 1.0

    in_maps = []
    for core in range(NCORES):
        b, hg = core // 4, core % 4
        r0 = hg * HPC * D          # first q/k/v row offset within each 1024
        r1 = r0 + HPC * D
        wqk = np.concatenate([w_attn[r0:r1, :], w_attn[C + r0:C + r1, :]], 0)
        in_maps.append({
            "xT": np.ascontiguousarray(x[b].T).astype(BF16),
            "wqkT": np.ascontiguousarray(wqk.T).astype(BF16),
            "wvT": np.ascontiguousarray(w_attn[2 * C + r0:2 * C + r1, :].T).astype(BF16),
            "wpT": np.ascontiguousarray(w_proj[r0:r1, :].T).astype(BF16),
            "bqk": np.concatenate([b_attn[r0:r1], b_attn[C + r0:C + r1]])
                     .reshape(FQK, 1).astype(np.float32),
            "bv": b_attn[2 * C + r0:2 * C + r1].reshape(1, FV).astype(BF16),
            "bp": b_proj[r0:r1].reshape(1, FV).astype(BF16),
            "maskb": mask,
            "onesr": np.ones((1, 64), dtype=np.float32),
        })
    return in_maps


def kernel(x, w_attn, b_attn, w_proj, b_proj, _trace=False, _trace_kwargs=None):
    x = np.asarray(x, dtype=np.float32)
    w_attn = np.asarray(w_attn, dtype=np.float32)
    b_attn = np.asarray(b_attn, dtype=np.float32)
    w_proj = np.asarray(w_proj, dtype=np.float32)
    b_proj = np.asarray(b_proj, dtype=np.float32)

    if "nc" not in _CACHE:
        _CACHE["nc"] = _build_kernel()
    nc = _CACHE["nc"]

    in_maps = _shard_inputs(x, w_attn, b_attn, w_proj, b_proj)
    res = run_bass_kernel_spmd(nc, in_maps, core_ids=list(range(NCORES)),
                               trace=_trace, **(_trace_kwargs or {}))
    _CACHE["last_result"] = res

    out = np.empty((B, T, C), dtype=np.float32)
    for core in range(NCORES):
        b, hg = core // 4, core % 4
        out[b, :, hg * FV:(hg + 1) * FV] = res.results[core]["out"]
    return out


# revision 9
# speedup vs baseline: 1.7573x; 1.0349x over previous
"""Causal self-attention (B=2, T=2048, C=1024, H=16) on 8 TRN2 NeuronCores.

Sharding: data parallel over batch (2) x tensor parallel over heads (4 groups
of 4 heads). Each core computes qkv + attention for its 4 heads of one batch,
normalized attention outputs are AllGathered (chunked over t, overlapped with
attention) within each batch group of 4 cores, and each core then computes a
256-column slice of the output projection. The host concatenates the column
slices (pure gather, no reduction).
"""
import numpy as np
import ml_dtypes

import concourse.bass as bass
import concourse.tile as tile
from concourse import bacc, mybir
from concourse.bass_utils import run_bass_kernel_spmd

BF16 = ml_dtypes.bfloat16

B, T, C, H, D = 2, 2048, 1024, 16, 64
NCORES = 8
HPC = 4              # heads per core
FQK = 2 * HPC * D    # 512 rows of q+k per core
FV = HPC * D         # 256 rows of v per core
CT = C // 128        # 8 contraction tiles
TC5 = T // 512       # 4 t-chunks of 512
SB = T // 128        # 16 s-blocks of 128
SCALE = 1.0 / 8.0    # 1/sqrt(D)

_CACHE = {}


def _build_kernel():
    nc = bacc.Bacc("TRN2", target_bir_lowering=False, debug=False,
                   num_devices=NCORES)
    dt = mybir.dt
    f32, bf16 = dt.float32, dt.bfloat16

    xT = nc.dram_tensor("xT", [C, T], bf16, kind="ExternalInput").ap()
    wqkT = nc.dram_tensor("wqkT", [C, FQK], bf16, kind="ExternalInput").ap()
    wvT = nc.dram_tensor("wvT", [C, FV], bf16, kind="ExternalInput").ap()
    wpT = nc.dram_tensor("wpT", [C, FV], bf16, kind="ExternalInput").ap()
    bqk = nc.dram_tensor("bqk", [FQK, 1], f32, kind="ExternalInput").ap()
    bv = nc.dram_tensor("bv", [1, FV], bf16, kind="ExternalInput").ap()
    bp = nc.dram_tensor("bp", [1, FV], bf16, kind="ExternalInput").ap()
    maskb = nc.dram_tensor("maskb", [128, 896], bf16, kind="ExternalInput").ap()
    onesr = nc.dram_tensor("onesr", [1, 64], mybir.dt.float32r,
                           kind="ExternalInput").ap()
    out = nc.dram_tensor("out", [T, FV], f32, kind="ExternalOutput").ap()

    with tile.TileContext(nc) as tc:
        with (
            tc.tile_pool(name="persist", bufs=1) as pp,
            tc.tile_pool(name="work", bufs=4) as wp,
            tc.tile_pool(name="attT", bufs=6) as ap_pool,
            tc.tile_pool(name="outsb", bufs=3) as op,
            tc.tile_pool(name="ps_qk", bufs=2, space="PSUM") as ps_qk,
            tc.tile_pool(name="ps_y", bufs=2, space="PSUM") as ps_y,
            tc.tile_pool(name="ps_bc", bufs=1, space="PSUM") as ps_bc,
            tc.tile_pool(name="ps_mm", bufs=1, space="PSUM") as ps_mm,
            tc.tile_pool(name="dram", bufs=1, space="DRAM") as dram,
        ):
            # ---- load constants / inputs to SBUF (split for pipelining) ----
            wqk_s = pp.tile([128, CT, FQK], bf16, tag="wqk")
            xT_s = pp.tile([128, CT, T], bf16, tag="xT")
            wv_s = pp.tile([128, CT, FV], bf16, tag="wv")
            xTr = xT.rearrange("(n p) t -> p n t", p=128)
            wqkr = wqkT.rearrange("(n p) f -> p n f", p=128)
            wvr = wvT.rearrange("(n p) f -> p n f", p=128)
            for ci in range(CT):
                nc.sync.dma_start(wqk_s[:, ci, :], wqkr[:, ci, :])
                nc.sync.dma_start(xT_s[:, ci, :], xTr[:, ci, :])
                nc.sync.dma_start(wv_s[:, ci, :], wvr[:, ci, :])
            wp_s = pp.tile([128, CT, FV], bf16, tag="wp")
            nc.sync.dma_start(wp_s[:], wpT.rearrange("(n p) f -> p n f", p=128))
            bqk_s = pp.tile([128, 4], f32, tag="bqk")
            nc.sync.dma_start(bqk_s[:], bqk.rearrange("(n p) o -> p (n o)", p=128))
            bv_s = pp.tile([1, FV], bf16, tag="bv")
            nc.sync.dma_start(bv_s[:], bv[:])
            bp_s = pp.tile([1, FV], bf16, tag="bp")
            nc.sync.dma_start(bp_s[:], bp[:])
            mask_s = pp.tile([128, 896], bf16, tag="mask")
            nc.sync.dma_start(mask_s[:], maskb[:])
            ones16 = pp.tile([1, 128], bf16, tag="ones16")
            nc.vector.memset(ones16[:], 1.0)
            ones32 = pp.tile([1, 64], mybir.dt.float32r, tag="ones32")
            nc.sync.dma_start(ones32[:], onesr[:])
            # broadcast bias rows to all 128 partitions once (K=1 matmuls)
            bv_bc = pp.tile([128, FV], f32, tag="bv_bc")
            bp_bc = pp.tile([128, FV], f32, tag="bp_bc")
            for row, bc_t in ((bv_s, bv_bc), (bp_s, bp_bc)):
                psb = ps_mm.tile([128, FV], f32, tag="mm", name="ps_bias")
                nc.tensor.matmul(psb[:], ones16[0:1, :], row[0:1, :],
                                 start=True, stop=True)
                nc.vector.tensor_copy(bc_t[:], psb[:])

            # ---- qkT / v chunk emitters (interleaved into attention loop) ----
            qkT_s = [pp.tile([128, T], bf16, tag=f"qkT{fc}", name=f"qkT{fc}")
                     for fc in range(4)]
            vaug = pp.tile([128, SB, HPC * 65], bf16, tag="vaug")
            nc.vector.memset(vaug[:], 1.0)

            def qkT_chunk(t5):
                for fc in range(4):
                    ps = ps_qk.tile([128, 512], f32, tag="qk", name="ps_qkv")
                    for ci in range(CT):
                        nc.tensor.matmul(
                            ps[:],
                            wqk_s[:, ci, fc * 128:(fc + 1) * 128],
                            xT_s[:, ci, t5 * 512:(t5 + 1) * 512],
                            start=(ci == 0), stop=(ci == CT - 1),
                        )
                    nc.vector.tensor_scalar_add(
                        qkT_s[fc][:, t5 * 512:(t5 + 1) * 512], ps[:],
                        bqk_s[:, fc:fc + 1],
                    )

            def v_chunk(t5):
                for tb in range(4 * t5, 4 * t5 + 4):
                    ps = ps_mm.tile([128, FV], f32, tag="mm", name="ps_v")
                    for ci in range(CT):
                        nc.tensor.matmul(
                            ps[:],
                            xT_s[:, ci, tb * 128:(tb + 1) * 128],
                            wv_s[:, ci, :],
                            start=(ci == 0), stop=(ci == CT - 1),
                        )
                    dst = vaug[:, tb, :].rearrange("p (h x) -> p h x", h=HPC)[:, :, 0:64]
                    src = ps[:].rearrange("p (h x) -> p h x", h=HPC)
                    bias = bv_bc[:].rearrange("p (h x) -> p h x", h=HPC)
                    nc.vector.scalar_tensor_tensor(
                        dst, src, 1.0, bias,
                        op0=mybir.AluOpType.mult, op1=mybir.AluOpType.add,
                    )

            # ---- attention, t-chunk major; AG + proj pipelined behind it ----
            ag_in, ag_out, yf = {}, {}, {}

            def proj_chunk(t5):
                yft = yf[t5]
                for tq in range(4):
                    tb = t5 * 4 + tq
                    pso = ps_mm.tile([128, FV], f32, tag="mm", name="ps_o")
                    for ci in range(CT):
                        nc.tensor.matmul(
                            pso[:],
                            yft[:, ci, tq * 128:(tq + 1) * 128],
                            wp_s[:, ci, :],
                            start=(ci == 0), stop=(ci == CT - 1),
                        )
                    osb = op.tile([128, FV], f32, tag="osb", name="osb")
                    nc.vector.tensor_add(osb[:], pso[:], bp_bc[:])
                    nc.sync.dma_start(out[tb * 128:(tb + 1) * 128, :], osb[:])

            qkT_chunk(0)
            v_chunk(0)
            for t5 in range(TC5):
                if t5 + 1 < TC5:
                    qkT_chunk(t5 + 1)
                    v_chunk(t5 + 1)
                live = 4 * (t5 + 1)
                ag_in[t5] = dram.tile([FV, 512], bf16, tag=f"agin{t5}",
                                      name=f"agin{t5}")
                ag_out[t5] = dram.tile([4 * FV, 512], bf16, tag=f"agout{t5}",
                                       name=f"agout{t5}")
                for pair in range(2):
                    q_fc, k_fc = pair, 2 + pair
                    ypsA = ps_y.tile([65, 512], f32, tag="y", name="ypsA")
                    ypsB = ps_y.tile([65, 512], f32, tag="y", name="ypsB")
                    for sb in range(live):
                        ps = ps_qk.tile([128, 1024], f32, tag="qk", name="ps_s")
                        for hh in range(2):
                            lo, hi = 64 * hh, 64 * (hh + 1)
                            nc.tensor.matmul(
                                ps[:, hh * 512:(hh + 1) * 512],
                                qkT_s[k_fc][lo:hi, sb * 128:(sb + 1) * 128],
                                qkT_s[q_fc][lo:hi, t5 * 512:(t5 + 1) * 512],
                                start=True, stop=True,
                            )
                        a = ap_pool.tile([128, 1024], bf16, tag="attT",
                                         name="attT")
                        nc.scalar.activation(
                            a[:], ps[:],
                            mybir.ActivationFunctionType.Exp, scale=SCALE,
                        )
                        off = sb * 128 - t5 * 512
                        if off >= 0:  # diagonal block: mask cols [0, off+128)
                            w = off + 128
                            msl = mask_s[:, 384 - off:384 - off + w]
                            nc.vector.tensor_mul(a[:, 0:w], a[:, 0:w], msl)
                            nc.vector.tensor_mul(a[:, 512:512 + w],
                                                 a[:, 512:512 + w], msl)
                        for hh, yps in ((0, ypsA), (1, ypsB)):
                            h = pair * 2 + hh
                            nc.tensor.matmul(
                                yps[:],
                                vaug[:, sb, h * 65:(h + 1) * 65],
                                a[:, hh * 512:(hh + 1) * 512],
                                start=(sb == 0), stop=(sb == live - 1),
                            )
                    # normalize: y / denom (denom = row 64 via ones column)
                    for hh, yps in ((0, ypsA), (1, ypsB)):
                        h = pair * 2 + hh
                        den = wp.tile([1, 512], mybir.dt.float32r, tag="den",
                                      name="den")
                        nc.vector.tensor_copy(den[:], yps[64:65, :])
                        bc = ps_bc.tile([64, 512], f32, tag="bc", name="bc")
                        nc.tensor.matmul(bc[:], ones32[0:1, :], den[:],
                                         start=True, stop=True)
                        r = wp.tile([64, 512], f32, tag="recip", name="recip")
                        nc.vector.reciprocal_approx_fast(r[:], bc[:])
                        yn = wp.tile([64, 512], bf16, tag="yn", name="yn")
                        nc.vector.tensor_mul(yn[:], yps[0:64, :], r[:])
                        nc.sync.dma_start(ag_in[t5][h * 64:(h + 1) * 64, :], yn[:])
                nc.gpsimd.collective_compute(
                    "AllGather", mybir.AluOpType.bypass,
                    replica_groups=[[0, 1, 2, 3], [4, 5, 6, 7]],
                    ins=[ag_in[t5][:].opt()], outs=[ag_out[t5][:].opt()],
                )
                yf[t5] = pp.tile([128, CT, 512], bf16, tag="yf", bufs=2,
                                 name=f"yf{t5}")
                nc.gpsimd.dma_start(
                    yf[t5][:], ag_out[t5][:].rearrange("(n p) t -> p n t", p=128))
                if t5 >= 2:
                    proj_chunk(t5 - 2)
            proj_chunk(2)
            proj_chunk(3)

    nc.compile()
    return nc


def _shard_inputs(x, w_attn, b_attn, w_proj, b_proj):
    mask = np.zeros((128, 896), dtype=BF16)
    for p in range(128):
        mask[p, 384 + p:] = 1.0

    in_maps = []
    for core in range(NCORES):
        b, hg = core // 4, core % 4
        r0 = hg * HPC * D          # first q/k/v row offset within each 1024
        r1 = r0 + HPC * D
        wqk = np.concatenate([w_attn[r0:r1, :], w_attn[C + r0:C + r1, :]], 0)
        in_maps.append({
            "xT": np.ascontiguousarray(x[b].T).astype(BF16),
            "wqkT": np.ascontiguousarray(wqk.T).astype(BF16),
            "wvT": np.ascontiguousarray(w_attn[2 * C + r0:2 * C + r1, :].T).astype(BF16),
            "wpT": np.ascontiguousarray(w_proj[r0:r1, :].T).astype(BF16),
            "bqk": np.concatenate([b_attn[r0:r1], b_attn[C + r0:C + r1]])
                     .reshape(FQK, 1).astype(np.float32),
            "bv": b_attn[2 * C + r0:2 * C + r1].reshape(1, FV).astype(BF16),
            "bp": b_proj[r0:r1].reshape(1, FV).astype(BF16),
            "maskb": mask,
            "onesr": np.ones((1, 64), dtype=np.float32),
        })
    return in_maps


def kernel(x, w_attn, b_attn, w_proj, b_proj, _trace=False, _trace_kwargs=None):
    x = np.asarray(x, dtype=np.float32)
    w_attn = np.asarray(w_attn, dtype=np.float32)
    b_attn = np.asarray(b_attn, dtype=np.float32)
    w_proj = np.asarray(w_proj, dtype=np.float32)
    b_proj = np.asarray(b_proj, dtype=np.float32)

    if "nc" not in _CACHE:
        _CACHE["nc"] = _build_kernel()
    nc = _CACHE["nc"]

    in_maps = _shard_inputs(x, w_attn, b_attn, w_proj, b_proj)
    res = run_bass_kernel_spmd(nc, in_maps, core_ids=list(range(NCORES)),
                               trace=_trace, **(_trace_kwargs or {}))
    _CACHE["last_result"] = res

    out = np.empty((B, T, C), dtype=np.float32)
    for core in range(NCORES):
        b, hg = core // 4, core % 4
        out[b, :, hg * FV:(hg + 1) * FV] = res.results[core]["out"]
    return out


# revision 10
# speedup vs baseline: 1.9909x; 1.1329x over previous
"""Causal self-attention (B=2, T=2048, C=1024, H=16) on 8 TRN2 NeuronCores.

Sharding: data parallel over batch (2) x tensor parallel over heads (4 groups
of 4 heads). Each core computes qkv + attention for its 4 heads of one batch,
normalized attention outputs are AllGathered (chunked over t, overlapped with
attention) within each batch group of 4 cores, and each core then computes a
256-column slice of the output projection. The host concatenates the column
slices (pure gather, no reduction).
"""
import numpy as np
import ml_dtypes

import concourse.bass as bass
import concourse.tile as tile
from concourse import bacc, mybir
from concourse.bass_utils import run_bass_kernel_spmd

BF16 = ml_dtypes.bfloat16

B, T, C, H, D = 2, 2048, 1024, 16, 64
NCORES = 8
HPC = 4              # heads per core
FQK = 2 * HPC * D    # 512 rows of q+k per core
FV = HPC * D         # 256 rows of v per core
CT = C // 128        # 8 contraction tiles
TC5 = T // 512       # 4 t-chunks of 512
SB = T // 128        # 16 s-blocks of 128
SCALE = 1.0 / 8.0    # 1/sqrt(D)

_CACHE = {}


def _build_kernel():
    nc = bacc.Bacc("TRN2", target_bir_lowering=False, debug=False,
                   num_devices=NCORES)
    dt = mybir.dt
    f32, bf16 = dt.float32, dt.bfloat16

    xT = nc.dram_tensor("xT", [C, T], bf16, kind="ExternalInput").ap()
    wqkT = nc.dram_tensor("wqkT", [C, FQK], bf16, kind="ExternalInput").ap()
    wvT = nc.dram_tensor("wvT", [C, FV], bf16, kind="ExternalInput").ap()
    wpT = nc.dram_tensor("wpT", [C, FV], bf16, kind="ExternalInput").ap()
    bqk = nc.dram_tensor("bqk", [FQK, 1], f32, kind="ExternalInput").ap()
    bv = nc.dram_tensor("bv", [1, FV], bf16, kind="ExternalInput").ap()
    bp = nc.dram_tensor("bp", [1, FV], bf16, kind="ExternalInput").ap()
    maskb = nc.dram_tensor("maskb", [128, 896], bf16, kind="ExternalInput").ap()
    onesr = nc.dram_tensor("onesr", [1, 64], mybir.dt.float32r,
                           kind="ExternalInput").ap()
    out = nc.dram_tensor("out", [T, FV], f32, kind="ExternalOutput").ap()

    with tile.TileContext(nc) as tc:
        with (
            tc.tile_pool(name="persist", bufs=1) as pp,
            tc.tile_pool(name="work", bufs=4) as wp,
            tc.tile_pool(name="attT", bufs=6) as ap_pool,
            tc.tile_pool(name="outsb", bufs=3) as op,
            tc.tile_pool(name="ps_qk", bufs=2, space="PSUM") as ps_qk,
            tc.tile_pool(name="ps_y", bufs=2, space="PSUM") as ps_y,
            tc.tile_pool(name="ps_bc", bufs=1, space="PSUM") as ps_bc,
            tc.tile_pool(name="ps_mm", bufs=1, space="PSUM") as ps_mm,
            tc.tile_pool(name="dram", bufs=1, space="DRAM") as dram,
        ):
            # ---- load constants / inputs to SBUF (split for pipelining) ----
            wqk_s = pp.tile([128, CT, FQK], bf16, tag="wqk")
            xT_s = pp.tile([128, CT, T], bf16, tag="xT")
            wv_s = pp.tile([128, CT, FV], bf16, tag="wv")
            xTr = xT.rearrange("(n p) t -> p n t", p=128)
            wqkr = wqkT.rearrange("(n p) f -> p n f", p=128)
            wvr = wvT.rearrange("(n p) f -> p n f", p=128)
            for ci in range(CT):
                nc.sync.dma_start(wqk_s[:, ci, :], wqkr[:, ci, :])
                nc.sync.dma_start(xT_s[:, ci, :], xTr[:, ci, :])
                nc.sync.dma_start(wv_s[:, ci, :], wvr[:, ci, :])
            wp_s = pp.tile([128, CT, FV], bf16, tag="wp")
            nc.sync.dma_start(wp_s[:], wpT.rearrange("(n p) f -> p n f", p=128))
            bqk_s = pp.tile([128, 4], f32, tag="bqk")
            nc.sync.dma_start(bqk_s[:], bqk.rearrange("(n p) o -> p (n o)", p=128))
            bv_s = pp.tile([1, FV], bf16, tag="bv")
            nc.sync.dma_start(bv_s[:], bv[:])
            bp_s = pp.tile([1, FV], bf16, tag="bp")
            nc.sync.dma_start(bp_s[:], bp[:])
            mask_s = pp.tile([128, 896], bf16, tag="mask")
            nc.sync.dma_start(mask_s[:], maskb[:])
            ones16 = pp.tile([1, 128], bf16, tag="ones16")
            nc.vector.memset(ones16[:], 1.0)
            ones32 = pp.tile([1, 64], mybir.dt.float32r, tag="ones32")
            nc.sync.dma_start(ones32[:], onesr[:])
            # broadcast bias rows to all 128 partitions once (K=1 matmuls)
            bv_bc = pp.tile([128, FV], f32, tag="bv_bc")
            bp_bc = pp.tile([128, FV], f32, tag="bp_bc")
            for row, bc_t in ((bv_s, bv_bc), (bp_s, bp_bc)):
                psb = ps_mm.tile([128, FV], f32, tag="mm", name="ps_bias")
                nc.tensor.matmul(psb[:], ones16[0:1, :], row[0:1, :],
                                 start=True, stop=True)
                nc.vector.tensor_copy(bc_t[:], psb[:])

            # ---- qkT / v chunk emitters (interleaved into attention loop) ----
            qkT_s = [pp.tile([128, T], bf16, tag=f"qkT{fc}", name=f"qkT{fc}")
                     for fc in range(4)]
            vaug = pp.tile([128, SB, HPC * 65], bf16, tag="vaug")
            nc.vector.memset(vaug[:], 1.0)

            def qkT_chunk(t5):
                for fc in range(4):
                    ps = ps_qk.tile([128, 512], f32, tag="qk", name="ps_qkv")
                    for ci in range(CT):
                        nc.tensor.matmul(
                            ps[:],
                            wqk_s[:, ci, fc * 128:(fc + 1) * 128],
                            xT_s[:, ci, t5 * 512:(t5 + 1) * 512],
                            start=(ci == 0), stop=(ci == CT - 1),
                        )
                    nc.vector.tensor_scalar_add(
                        qkT_s[fc][:, t5 * 512:(t5 + 1) * 512], ps[:],
                        bqk_s[:, fc:fc + 1],
                    )

            def v_chunk(t5):
                for tb in range(4 * t5, 4 * t5 + 4):
                    ps = ps_mm.tile([128, FV], f32, tag="mm", name="ps_v")
                    for ci in range(CT):
                        nc.tensor.matmul(
                            ps[:],
                            xT_s[:, ci, tb * 128:(tb + 1) * 128],
                            wv_s[:, ci, :],
                            start=(ci == 0), stop=(ci == CT - 1),
                        )
                    dst = vaug[:, tb, :].rearrange("p (h x) -> p h x", h=HPC)[:, :, 0:64]
                    src = ps[:].rearrange("p (h x) -> p h x", h=HPC)
                    bias = bv_bc[:].rearrange("p (h x) -> p h x", h=HPC)
                    nc.vector.scalar_tensor_tensor(
                        dst, src, 1.0, bias,
                        op0=mybir.AluOpType.mult, op1=mybir.AluOpType.add,
                    )

            # ---- attention, t-chunk major; AG + proj pipelined behind it ----
            ag_in, ag_out, yf = {}, {}, {}

            def proj_chunk(t5):
                yft = yf[t5]
                for tq in range(4):
                    tb = t5 * 4 + tq
                    pso = ps_mm.tile([128, FV], f32, tag="mm", name="ps_o")
                    for ci in range(CT):
                        par, cc = ci % 2, ci // 2
                        nc.tensor.matmul(
                            pso[:],
                            yft[:, par, cc, tq * 128:(tq + 1) * 128],
                            wp_s[:, ci, :],
                            start=(ci == 0), stop=(ci == CT - 1),
                        )
                    osb = op.tile([128, FV], f32, tag="osb", name="osb")
                    nc.vector.tensor_add(osb[:], pso[:], bp_bc[:])
                    nc.sync.dma_start(out[tb * 128:(tb + 1) * 128, :], osb[:])

            qkT_chunk(0)
            v_chunk(0)
            for t5 in range(TC5):
                if t5 + 1 < TC5:
                    qkT_chunk(t5 + 1)
                    v_chunk(t5 + 1)
                live = 4 * (t5 + 1)
                for pr in range(2):
                    ag_in[(t5, pr)] = dram.tile([128, 512], bf16,
                                                tag=f"agin{t5}_{pr}",
                                                name=f"agin{t5}_{pr}")
                    ag_out[(t5, pr)] = dram.tile([512, 512], bf16,
                                                 tag=f"agout{t5}_{pr}",
                                                 name=f"agout{t5}_{pr}")
                yf[t5] = pp.tile([128, 2, CT // 2, 512], bf16, tag="yf", bufs=2,
                                 name=f"yf{t5}")
                for pair in range(2):
                    q_fc, k_fc = pair, 2 + pair
                    ypsA = ps_y.tile([65, 512], f32, tag="y", name="ypsA")
                    ypsB = ps_y.tile([65, 512], f32, tag="y", name="ypsB")
                    for sb in range(live):
                        ps = ps_qk.tile([128, 1024], f32, tag="qk", name="ps_s")
                        for hh in range(2):
                            lo, hi = 64 * hh, 64 * (hh + 1)
                            nc.tensor.matmul(
                                ps[:, hh * 512:(hh + 1) * 512],
                                qkT_s[k_fc][lo:hi, sb * 128:(sb + 1) * 128],
                                qkT_s[q_fc][lo:hi, t5 * 512:(t5 + 1) * 512],
                                start=True, stop=True,
                            )
                        a = ap_pool.tile([128, 1024], bf16, tag="attT",
                                         name="attT")
                        nc.scalar.activation(
                            a[:], ps[:],
                            mybir.ActivationFunctionType.Exp, scale=SCALE,
                        )
                        off = sb * 128 - t5 * 512
                        if off >= 0:  # diagonal block: mask cols [0, off+128)
                            w = off + 128
                            msl = mask_s[:, 384 - off:384 - off + w]
                            nc.vector.tensor_mul(a[:, 0:w], a[:, 0:w], msl)
                            nc.vector.tensor_mul(a[:, 512:512 + w],
                                                 a[:, 512:512 + w], msl)
                        for hh, yps in ((0, ypsA), (1, ypsB)):
                            h = pair * 2 + hh
                            nc.tensor.matmul(
                                yps[:],
                                vaug[:, sb, h * 65:(h + 1) * 65],
                                a[:, hh * 512:(hh + 1) * 512],
                                start=(sb == 0), stop=(sb == live - 1),
                            )
                    # normalize: y / denom (denom = row 64 via ones column)
                    for hh, yps in ((0, ypsA), (1, ypsB)):
                        h = pair * 2 + hh
                        den = wp.tile([1, 512], mybir.dt.float32r, tag="den",
                                      name="den")
                        nc.vector.tensor_copy(den[:], yps[64:65, :])
                        bc = ps_bc.tile([64, 512], f32, tag="bc", name="bc")
                        nc.tensor.matmul(bc[:], ones32[0:1, :], den[:],
                                         start=True, stop=True)
                        r = wp.tile([64, 512], f32, tag="recip", name="recip")
                        nc.vector.reciprocal_approx_fast(r[:], bc[:])
                        yn = wp.tile([64, 512], bf16, tag="yn", name="yn")
                        nc.vector.tensor_mul(yn[:], yps[0:64, :], r[:])
                        nc.sync.dma_start(
                            ag_in[(t5, pair)][hh * 64:(hh + 1) * 64, :], yn[:])
                    nc.gpsimd.collective_compute(
                        "AllGather", mybir.AluOpType.bypass,
                        replica_groups=[[0, 1, 2, 3], [4, 5, 6, 7]],
                        ins=[ag_in[(t5, pair)][:].opt()],
                        outs=[ag_out[(t5, pair)][:].opt()],
                    )
                    nc.gpsimd.dma_start(
                        yf[t5][:, pair, :, :],
                        ag_out[(t5, pair)][:].rearrange("(n p) t -> p n t", p=128))
                if t5 >= 2:
                    proj_chunk(t5 - 2)
            proj_chunk(2)
            proj_chunk(3)

    nc.compile()
    return nc


def _shard_inputs(x, w_attn, b_attn, w_proj, b_proj):
    mask = np.zeros((128, 896), dtype=BF16)
    for p in range(128):
        mask[p, 384 + p:] = 1.0

    in_maps = []
    for core in range(NCORES):
        b, hg = core // 4, core % 4
        r0 = hg * HPC * D          # first q/k/v row offset within each 1024
        r1 = r0 + HPC * D
        wqk = np.concatenate([w_attn[r0:r1, :], w_attn[C + r0:C + r1, :]], 0)
        in_maps.append({
            "xT": np.ascontiguousarray(x[b].T).astype(BF16),
            "wqkT": np.ascontiguousarray(wqk.T).astype(BF16),
            "wvT": np.ascontiguousarray(w_attn[2 * C + r0:2 * C + r1, :].T).astype(BF16),
            "wpT": np.ascontiguousarray(w_proj[r0:r1, :].T).astype(BF16),
            "bqk": np.concatenate([b_attn[r0:r1], b_attn[C + r0:C + r1]])
                     .reshape(FQK, 1).astype(np.float32),
            "bv": b_attn[2 * C + r0:2 * C + r1].reshape(1, FV).astype(BF16),
            "bp": b_proj[r0:r1].reshape(1, FV).astype(BF16),
            "maskb": mask,
            "onesr": np.ones((1, 64), dtype=np.float32),
        })
    return in_maps


def kernel(x, w_attn, b_attn, w_proj, b_proj, _trace=False, _trace_kwargs=None):
    x = np.asarray(x, dtype=np.float32)
    w_attn = np.asarray(w_attn, dtype=np.float32)
    b_attn = np.asarray(b_attn, dtype=np.float32)
    w_proj = np.asarray(w_proj, dtype=np.float32)
    b_proj = np.asarray(b_proj, dtype=np.float32)

    if "nc" not in _CACHE:
        _CACHE["nc"] = _build_kernel()
    nc = _CACHE["nc"]

    in_maps = _shard_inputs(x, w_attn, b_attn, w_proj, b_proj)
    res = run_bass_kernel_spmd(nc, in_maps, core_ids=list(range(NCORES)),
                               trace=_trace, **(_trace_kwargs or {}))
    _CACHE["last_result"] = res

    out = np.empty((B, T, C), dtype=np.float32)
    for core in range(NCORES):
        b, hg = core // 4, core % 4
        out[b, :, hg * FV:(hg + 1) * FV] = res.results[core]["out"]
    return out
